# revision 1
# baseline (speedup 1.0000x reference)
"""Trainium2 Bass kernel for nn_Mk1_91036126806096.

Shared-weight LSTM (3 units, all-sigmoid activations) over [192 folded
sequences x T=4096 x 64 features], followed by a 4-unit dense layer with
sigmoid.  Data-parallel over 8 NeuronCores (8 original batch elements,
i.e. 24 folded sequences, per core).

The sequential scan is replaced by a Picard fixed-point iteration: given
gate values the c-recurrence c_t = f_t*c_{t-1} + i_t*g_t is linear and
runs in one DVE tensor_tensor_scan instruction per 512-step chunk; the
gates are recomputed from the lagged h trajectory each sweep.  The
iteration contracts by ~10x per sweep (verified vs the sequential
reference), so K sweeps reach the fp32 noise floor for K >= 8.

Per-core layout: "lane" L = 3*s + u for folded sequence s = 3*b + c
(b = local batch 0..7, c = feature chunk 0..2) and unit u.  Everything in
phase 2 lives on lanes 0..71 with time (and the 4 gates, as 4 blocks)
along the free dimension, so gate slicing is free-dim only.

Phase-2 matmuls run in float32r (single-pass PE, ~11-bit operand
rounding); phase 1 (column-packed, fp32r cannot column-tile) and the
final dense matmul stay fp32.  Emulated end-to-end error of this mix vs
the fp32 reference: ~1.8e-5 absolute on outputs in (0,1).
"""

import numpy as np

UNITS = 3
GATES = 4
B_FULL = 64
T_FULL = 4096
F = 64
N_CORES = 8
NB = 8                 # batch elements per core
NS = NB * 3            # folded sequences per core
L = NS * UNITS         # lanes = 72
TC = 512               # time chunk (one PSUM bank of fp32)
K_ITERS = 8            # Picard sweeps
MM_R = True            # float32r matmuls for phases 1-2

_cache = {}
TRACE = False
_last_exec_ns = None


def _build_module(T, k_iters, mm_r, debug):
    import concourse.bass as bass
    import concourse.tile as tile
    from concourse import bacc, mybir

    f32 = mybir.dt.float32
    mmdt = mybir.dt.float32r if mm_r else f32
    AF = mybir.ActivationFunctionType
    OP = mybir.AluOpType
    NCH = T // TC
    HT = T // 2

    nc = bacc.Bacc("TRN2", target_bir_lowering=False, debug=debug)

    xt = nc.dram_tensor("xt", [NS, F, T], f32, kind="ExternalInput")
    w_d = nc.dram_tensor("w", [2 * F, 12], f32, kind="ExternalInput")
    iz_d = nc.dram_tensor("iz", [L + 1, GATES * L], mmdt, kind="ExternalInput")
    bdu_d = nc.dram_tensor("bdu", [L, GATES * L], mmdt, kind="ExternalInput")
    s3_d = nc.dram_tensor("s3", [L, 4 * NB], f32, kind="ExternalInput")
    bdv_d = nc.dram_tensor("bdv", [4 * NB, 1], f32, kind="ExternalInput")
    ones_d = nc.dram_tensor("ones1", [1, GATES * T], mmdt, kind="ExternalInput")
    zeros_d = nc.dram_tensor("zeros1", [L, 1 + T], mmdt, kind="ExternalInput")
    y_d = nc.dram_tensor("y", [4 * NB, T], f32, kind="ExternalOutput")

    with tile.TileContext(nc) as tc:
        with tc.tile_pool(name="const", bufs=1) as cp, \
             tc.tile_pool(name="persist", bufs=1) as pp:
            w_t = cp.tile([2 * F, 12], f32, tag="w")
            nc.sync.dma_start(w_t[:], w_d.ap())
            iz_t = cp.tile([L + 1, GATES * L], mmdt, tag="iz")
            nc.sync.dma_start(iz_t[:], iz_d.ap())
            bdu_t = cp.tile([L, GATES * L], mmdt, tag="bdu")
            nc.sync.dma_start(bdu_t[:], bdu_d.ap())
            s3_t = cp.tile([L, 4 * NB], f32, tag="s3")
            nc.sync.dma_start(s3_t[:], s3_d.ap())
            bdv_t = cp.tile([4 * NB, 1], f32, tag="bdv")
            nc.sync.dma_start(bdv_t[:], bdv_d.ap())

            zpre = pp.tile([L + 1, GATES * T], mmdt, tag="zpre")
            nc.sync.dma_start(zpre[L:L + 1, :], ones_d.ap())
            hA = pp.tile([L, 1 + T], mmdt, tag="hA")
            hB = pp.tile([L, 1 + T], mmdt, tag="hB")
            nc.sync.dma_start(hA[:, :], zeros_d.ap())
            nc.sync.dma_start(hB[:, 0:1], zeros_d.ap()[:, 0:1])

            # ---------------- Phase 1: zpre = x @ W ----------------
            # 4 seqs per PSUM tile via column-group packing; staging
            # holds the whole T so the scatter to zpre's (s,u)-major
            # layout is 16 large DMAs per group of 4 seqs.
            with tc.tile_pool(name="xp", bufs=2) as xp, \
                 tc.tile_pool(name="stgp", bufs=2) as stgp, \
                 tc.tile_pool(name="ps1", bufs=1, space="PSUM") as ps1p:
                pts = []
                for i in range(3):
                    pt = ps1p.tile([128, TC], f32, tag=f"p1b{i}")
                    nc.vector.memset(pt[:, :], 0.0)
                    pts.append(pt)
                it = 0
                for g in range(NS // 4):
                    stg = stgp.tile([108, T], mmdt, tag="stg")
                    for half in range(2):
                        xA = xp.tile([128, HT], f32, tag="xA")
                        xB = xp.tile([128, HT], f32, tag="xB")
                        nc.sync.dma_start(
                            xA[:], xt.ap()[4 * g:4 * g + 2, :,
                                           half * HT:(half + 1) * HT])
                        nc.sync.dma_start(
                            xB[:], xt.ap()[4 * g + 2:4 * g + 4, :,
                                           half * HT:(half + 1) * HT])
                        for j in range(NCH // 2):
                            pt = pts[it % 3]
                            for q in range(4):
                                xtile = xA if q < 2 else xB
                                r0 = (q % 2) * 64
                                nc.tensor.matmul(
                                    pt[32 * q:32 * q + 12, :],
                                    w_t[r0:r0 + 64, :],
                                    xtile[r0:r0 + 64, j * TC:(j + 1) * TC],
                                    start=True, stop=True,
                                    tile_position=(r0, 32 * q))
                            col = (half * (NCH // 2) + j) * TC
                            if it % 2 == 0:
                                nc.scalar.copy(stg[:, col:col + TC],
                                               pt[0:108, :])
                            else:
                                nc.vector.tensor_copy(stg[:, col:col + TC],
                                                      pt[0:108, :])
                            it += 1
                    for q in range(4):
                        s = 4 * g + q
                        for gt in range(GATES):
                            eng = nc.sync if (q + gt) % 2 == 0 else nc.scalar
                            eng.dma_start(
                                zpre[3 * s:3 * s + 3, gt * T:(gt + 1) * T],
                                stg[32 * q + 3 * gt:32 * q + 3 * gt + 3, :])

            # ---------------- Phase 2: Picard sweeps ----------------
            with tc.tile_pool(name="sp", bufs=3) as sp, \
                 tc.tile_pool(name="igp", bufs=2) as igp, \
                 tc.tile_pool(name="scp", bufs=2) as scp, \
                 tc.tile_pool(name="cpool", bufs=3) as cpl, \
                 tc.tile_pool(name="zps", bufs=2, space="PSUM") as zpsp:
                hbufs = [hA, hB]
                for k in range(k_iters):
                    hold = hbufs[k % 2]
                    hnew = hbufs[(k + 1) % 2]
                    c_prev = None
                    for j in range(NCH):
                        zps = zpsp.tile([L, GATES * TC], f32, tag="zps")
                        for gt in range(GATES):
                            nc.tensor.matmul(
                                zps[:, gt * TC:(gt + 1) * TC],
                                iz_t[:, gt * L:(gt + 1) * L],
                                zpre[:, gt * T + j * TC:gt * T + (j + 1) * TC],
                                start=True, stop=False, tile_position=(0, 0))
                            nc.tensor.matmul(
                                zps[:, gt * TC:(gt + 1) * TC],
                                bdu_t[:, gt * L:(gt + 1) * L],
                                hold[:, j * TC:(j + 1) * TC],
                                start=False, stop=True, tile_position=(0, 0))
                        s_t = sp.tile([L, GATES * TC], f32, tag="s")
                        nc.scalar.activation(s_t[:], zps[:, :], AF.Sigmoid)
                        ig = igp.tile([L, TC], f32, tag="ig")
                        nc.vector.tensor_tensor(
                            out=ig[:], in0=s_t[:, 0:TC],
                            in1=s_t[:, 2 * TC:3 * TC], op=OP.mult)
                        c_t = cpl.tile([L, TC], f32, tag="c")
                        init = 0.0 if j == 0 else c_prev[:, TC - 1:TC]
                        nc.vector.tensor_tensor_scan(
                            out=c_t[:], data0=s_t[:, TC:2 * TC], data1=ig[:],
                            initial=init, op0=OP.mult, op1=OP.add)
                        c_prev = c_t
                        sc_t = scp.tile([L, TC], f32, tag="sc")
                        nc.scalar.activation(sc_t[:], c_t[:], AF.Sigmoid)
                        nc.vector.tensor_tensor(
                            out=hnew[:, 1 + j * TC:1 + (j + 1) * TC],
                            in0=s_t[:, 3 * TC:4 * TC], in1=sc_t[:], op=OP.mult)

            # ---------------- Phase 3: dense + sigmoid (fp32) -------
            hfin = hbufs[k_iters % 2]
            hfin_f = hfin[:].bitcast(f32) if mm_r else hfin[:]
            with tc.tile_pool(name="yp", bufs=2) as yp, \
                 tc.tile_pool(name="ps3", bufs=2, space="PSUM") as ps3p:
                for j in range(NCH):
                    p3 = ps3p.tile([4 * NB, TC], f32, tag="p3")
                    nc.tensor.matmul(
                        p3[:, :], s3_t[:, :],
                        hfin_f[:, 1 + j * TC:1 + (j + 1) * TC],
                        start=True, stop=True, tile_position=(0, 0))
                    y_t = yp.tile([4 * NB, TC], f32, tag="y")
                    nc.scalar.activation(y_t[:], p3[:, :], AF.Sigmoid,
                                         bias=bdv_t[:, :])
                    nc.sync.dma_start(y_d.ap()[:, j * TC:(j + 1) * TC], y_t[:])

    nc.compile()
    return nc


def _rnd11(v):
    """Round to 11 explicit mantissa bits (what fp32r keeps of operands)."""
    u = np.ascontiguousarray(v, np.float32).view(np.int32)
    s = 23 - 11
    return (((u + (1 << (s - 1))) >> s) << s).astype(np.int32).view(np.float32)


def _host_consts(W, U, b, Wd, bd, T, mm_r):
    """Pack the small parameter matrices into the stationary layouts."""
    W = np.asarray(W, np.float32)
    U = np.asarray(U, np.float32)
    b = np.asarray(b, np.float32)
    Wd = np.asarray(Wd, np.float32)
    bd = np.asarray(bd, np.float32)

    iz = np.zeros((L + 1, GATES * L), np.float32)
    bdu = np.zeros((L, GATES * L), np.float32)
    for gt in range(GATES):
        blk = iz[:, gt * L:(gt + 1) * L]
        blk[0:L, :] = np.eye(L, dtype=np.float32)
        for s in range(NS):
            for u in range(UNITS):
                blk[L, 3 * s + u] = b[3 * gt + u]
        ublk = bdu[:, gt * L:(gt + 1) * L]
        for s in range(NS):
            for up in range(UNITS):
                for u in range(UNITS):
                    ublk[3 * s + up, 3 * s + u] = U[up, 3 * gt + u]
    s3 = np.zeros((L, 4 * NB), np.float32)
    for bb in range(NB):
        for c in range(3):
            for u in range(UNITS):
                for d in range(4):
                    s3[9 * bb + 3 * c + u, 4 * bb + d] = Wd[3 * c + u, d]
    bdv = np.tile(bd, NB).reshape(4 * NB, 1).astype(np.float32)
    ones = np.ones((1, GATES * T), np.float32)
    zeros = np.zeros((L, 1 + T), np.float32)
    if mm_r:
        iz, bdu = _rnd11(iz), _rnd11(bdu)
    W2 = np.concatenate([W, W], axis=0)
    return {"w": W2, "iz": iz, "bdu": bdu, "s3": s3, "bdv": bdv, "ones1": ones,
            "zeros1": zeros}


def _host_xt(inputs, T):
    """[B, T, 192] -> per-core [NS, F, T] with s = 3*b_local + c."""
    B = inputs.shape[0]
    x = np.asarray(inputs, np.float32).reshape(B, T, 3, F)
    x = np.ascontiguousarray(np.transpose(x, (0, 2, 3, 1)))  # [B, c, F, T]
    per_core = []
    for k in range(N_CORES):
        per_core.append(x[k * NB:(k + 1) * NB].reshape(NS, F, T))
    return per_core


def kernel(inputs, W, U, b, Wd, bd):
    from concourse.bass_utils import run_bass_kernel_spmd

    B, T, F3 = inputs.shape
    assert (B, T, F3) == (B_FULL, T_FULL, 192)

    key = (T, K_ITERS, MM_R)
    if key not in _cache:
        _cache[key] = _build_module(T, K_ITERS, MM_R, debug=False)
    nc = _cache[key]

    consts = _host_consts(W, U, b, Wd, bd, T, MM_R)
    xts = _host_xt(inputs, T)
    in_maps = [dict(consts, xt=xts[k]) for k in range(N_CORES)]

    global _last_exec_ns
    res = run_bass_kernel_spmd(nc, in_maps, list(range(N_CORES)), trace=TRACE)
    if res.exec_time_ns is not None:
        _last_exec_ns = res.exec_time_ns
    ys = [res.results[k]["y"] for k in range(N_CORES)]  # [32, T] each

    out = np.empty((B, T, 4), np.float32)
    for k in range(N_CORES):
        blk = ys[k].reshape(NB, 4, T)          # [b, d, t]
        out[k * NB:(k + 1) * NB] = np.transpose(blk, (0, 2, 1))
    return out



# revision 6
# speedup vs baseline: 1.6858x; 1.6858x over previous
"""Trainium2 Bass kernel for nn_Mk1_91036126806096.

Shared-weight LSTM (3 units, all-sigmoid activations) over [192 folded
sequences x T=4096 x 64 features], followed by a 4-unit dense layer with
sigmoid.  Data-parallel over 8 NeuronCores (8 original batch elements,
i.e. 24 folded sequences, per core).

The sequential scan is replaced by a Picard fixed-point iteration: given
gate values the c-recurrence c_t = f_t*c_{t-1} + i_t*g_t is linear and
runs in one DVE tensor_tensor_scan instruction per 512-step chunk; the
gates are recomputed from the lagged h trajectory each sweep.  The
iteration contracts ~10x per sweep; K=3 sweeps give ~2.7e-3 rel error
end-to-end in bf16 (threshold 2e-2).

Layout: lane L = 3*s + u for folded sequence s = 3*b_local + c and unit
u; everything in phase 2 lives on lanes 0..71 with time (and the 4
gates, as 4 blocks) along the free dimension.

Numerics: x, weights, gates, c and h are bf16 (PE streams bf16 at full
rate; DVE gets 2x modes); PSUM accumulation is fp32.  The bias b rides
as a 73rd all-ones row of zpre against a b-row in the identity
stationary (sweeps 2+) / as a per-partition activation bias (phase 1
and sweep 1).
"""

import numpy as np
import ml_dtypes

BF16 = ml_dtypes.bfloat16

UNITS = 3
GATES = 4
B_FULL = 64
T_FULL = 4096
F = 64
N_CORES = 8
NB = 8                 # batch elements per core
NS = NB * 3            # folded sequences per core
L = NS * UNITS         # lanes = 72
TC = 512               # time chunk (one PSUM bank of fp32)
K_ITERS = 3            # Picard sweeps
NGROUP = 3             # seq-pair groups in phase 1 (4 pairs each)

_cache = {}
TRACE = False
_last_exec_ns = None


def _build_module(T, k_iters, debug):
    import concourse.bass as bass
    import concourse.tile as tile
    from concourse import bacc, mybir

    f32 = mybir.dt.float32
    bf16 = mybir.dt.bfloat16
    AF = mybir.ActivationFunctionType
    OP = mybir.AluOpType
    NCH = T // TC
    HT = T // 2

    nc = bacc.Bacc("TRN2", target_bir_lowering=False, debug=debug)

    # x, per core: [NS, F, T] bf16 with s = 3*b_local + c
    xt = nc.dram_tensor("xt", [NS, F, T], bf16, kind="ExternalInput")
    # W2: block-diag [128, 24]: rows 0:64 -> cols 0:12 (seq a), rows
    # 64:128 -> cols 12:24 (seq b); column order gt*3+u.
    w2_d = nc.dram_tensor("w2", [2 * F, 24], bf16, kind="ExternalInput")
    # per-gate identity-with-bias-row stationaries [73, 72], 4 gates
    idb_d = nc.dram_tensor("idb", [L + 1, GATES * L], bf16, kind="ExternalInput")
    # block-diag U per gate [72, 4*72]
    bdu_d = nc.dram_tensor("bdu", [L, GATES * L], bf16, kind="ExternalInput")
    # dense stationary [72, 32] and bias [32, 1]
    s3_d = nc.dram_tensor("s3", [L, 4 * NB], bf16, kind="ExternalInput")
    bdv_d = nc.dram_tensor("bdv", [4 * NB, 1], f32, kind="ExternalInput")
    # phase-1 evacuation bias [128, 1] (b per (gt,u) row-block pattern)
    bev_d = nc.dram_tensor("bev", [128, 1], f32, kind="ExternalInput")
    # sweep-1 per-gate bias [72, 1] x 4 gates
    bg_d = nc.dram_tensor("bg", [L, GATES], f32, kind="ExternalInput")
    # all-ones bias row for zpre (engine memset can't start at partition 72)
    ones_d = nc.dram_tensor("ones1", [1, GATES * T], bf16, kind="ExternalInput")
    y_d = nc.dram_tensor("y", [4 * NB, T], f32, kind="ExternalOutput")

    with tile.TileContext(nc) as tc:
        with tc.tile_pool(name="const", bufs=1) as cp, \
             tc.tile_pool(name="persist", bufs=1) as pp:
            w2_t = cp.tile([2 * F, 24], bf16, tag="w2")
            nc.sync.dma_start(w2_t[:], w2_d.ap())
            idb_t = cp.tile([L + 1, GATES * L], bf16, tag="idb")
            nc.sync.dma_start(idb_t[:], idb_d.ap())
            bdu_t = cp.tile([L, GATES * L], bf16, tag="bdu")
            nc.sync.dma_start(bdu_t[:], bdu_d.ap())
            s3_t = cp.tile([L, 4 * NB], bf16, tag="s3")
            nc.sync.dma_start(s3_t[:], s3_d.ap())
            bdv_t = cp.tile([4 * NB, 1], f32, tag="bdv")
            nc.sync.dma_start(bdv_t[:], bdv_d.ap())
            bev_t = cp.tile([128, 1], f32, tag="bev")
            nc.sync.dma_start(bev_t[:], bev_d.ap())
            bg_t = cp.tile([L, GATES], f32, tag="bg")
            nc.sync.dma_start(bg_t[:], bg_d.ap())

            # zpre: [73, GATES*T] bf16; row 72 = ones (bias row)
            zpre = pp.tile([L + 1, GATES * T], bf16, tag="zpre")
            nc.sync.dma_start(zpre[L:L + 1, :], ones_d.ap())
            hA = pp.tile([L, 1 + T], bf16, tag="hA")
            hB = pp.tile([L, 1 + T], bf16, tag="hB")
            nc.vector.memset(hA[:, 0:1], 0.0)
            nc.vector.memset(hB[:, 0:1], 0.0)

            # ---------------- Phase 1: zpre = x @ W + b ----------------
            # 4 seq-pairs per PSUM tile via column tiling (out partitions
            # 32p..32p+24); ACT evacuates with per-partition bias; scatter
            # DMAs land the (gt,u)-row layout into zpre's gate-major rows.
            with tc.tile_pool(name="xp", bufs=3) as xp, \
                 tc.tile_pool(name="stgp", bufs=2) as stgp, \
                 tc.tile_pool(name="ps1", bufs=2, space="PSUM") as ps1p:
                for half in range(2):
                    for g in range(NGROUP):
                        xts = []
                        for p in range(4):
                            xtl = xp.tile([128, HT], bf16, tag=f"x{p}")
                            pr = 8 * g + 2 * p
                            nc.sync.dma_start(
                                xtl[:], xt.ap()[pr:pr + 2, :,
                                                half * HT:(half + 1) * HT])
                            xts.append(xtl)
                        stg = stgp.tile([128, HT], bf16, tag="stg")
                        for j in range(HT // TC):
                            pt = ps1p.tile([128, TC], f32, tag="p1")
                            for p in range(4):
                                nc.tensor.matmul(
                                    pt[32 * p:32 * p + 24, :],
                                    w2_t[:, :],
                                    xts[p][:, j * TC:(j + 1) * TC],
                                    start=True, stop=True,
                                    tile_position=(0, 32 * p))
                            nc.scalar.activation(
                                stg[:, j * TC:(j + 1) * TC], pt[:, :],
                                AF.Identity, bias=bev_t[:, :])
                        # scatter: 8 seqs x 4 gates -> zpre rows
                        for p in range(4):
                            for sl in range(2):
                                s = 8 * g + 2 * p + sl
                                for gt in range(GATES):
                                    r = 32 * p + 12 * sl + 3 * gt
                                    nc.scalar.dma_start(
                                        zpre[3 * s:3 * s + 3,
                                             gt * T + half * HT:
                                             gt * T + (half + 1) * HT],
                                        stg[r:r + 3, :])

            # ---------------- Phase 2: Picard sweeps ----------------
            hbufs = [hA, hB]
            with tc.tile_pool(name="sp", bufs=3) as sp, \
                 tc.tile_pool(name="igp", bufs=2) as igp, \
                 tc.tile_pool(name="scp", bufs=2) as scp, \
                 tc.tile_pool(name="cpool", bufs=3) as cpl, \
                 tc.tile_pool(name="zps", bufs=2, space="PSUM") as zpsp:
                for k in range(k_iters):
                    hold = hbufs[k % 2]
                    hnew = hbufs[(k + 1) % 2]
                    c_prev = None
                    for j in range(NCH):
                        s_t = sp.tile([L, GATES * TC], bf16, tag="s")
                        if k == 0:
                            # gates straight from zpre (h == 0)
                            for gt in range(GATES):
                                nc.scalar.activation(
                                    s_t[:, gt * TC:(gt + 1) * TC],
                                    zpre[0:L, gt * T + j * TC:
                                         gt * T + (j + 1) * TC],
                                    AF.Sigmoid, bias=bg_t[:, gt:gt + 1])
                        else:
                            zps = zpsp.tile([L, GATES * TC], f32, tag="zps")
                            for gt in range(GATES):
                                nc.tensor.matmul(
                                    zps[:, gt * TC:(gt + 1) * TC],
                                    idb_t[:, gt * L:(gt + 1) * L],
                                    zpre[:, gt * T + j * TC:
                                         gt * T + (j + 1) * TC],
                                    start=True, stop=False,
                                    tile_position=(0, 0))
                                nc.tensor.matmul(
                                    zps[:, gt * TC:(gt + 1) * TC],
                                    bdu_t[:, gt * L:(gt + 1) * L],
                                    hold[:, j * TC:(j + 1) * TC],
                                    start=False, stop=True,
                                    tile_position=(0, 0))
                            nc.scalar.activation(s_t[:], zps[:, :], AF.Sigmoid)
                        ig = igp.tile([L, TC], bf16, tag="ig")
                        nc.vector.tensor_tensor(
                            out=ig[:], in0=s_t[:, 0:TC],
                            in1=s_t[:, 2 * TC:3 * TC], op=OP.mult)
                        c_t = cpl.tile([L, TC], bf16, tag="c")
                        init = 0.0 if j == 0 else c_prev[:, TC - 1:TC]
                        nc.vector.tensor_tensor_scan(
                            out=c_t[:], data0=s_t[:, TC:2 * TC], data1=ig[:],
                            initial=init, op0=OP.mult, op1=OP.add)
                        c_prev = c_t
                        sc_t = scp.tile([L, TC], bf16, tag="sc")
                        nc.scalar.activation(sc_t[:], c_t[:], AF.Sigmoid)
                        nc.vector.tensor_tensor(
                            out=hnew[:, 1 + j * TC:1 + (j + 1) * TC],
                            in0=s_t[:, 3 * TC:4 * TC], in1=sc_t[:], op=OP.mult)

            # ---------------- Phase 3: dense + sigmoid -------
            hfin = hbufs[k_iters % 2]
            with tc.tile_pool(name="yp", bufs=2) as yp, \
                 tc.tile_pool(name="ps3", bufs=2, space="PSUM") as ps3p:
                for j in range(NCH):
                    p3 = ps3p.tile([4 * NB, TC], f32, tag="p3")
                    nc.tensor.matmul(
                        p3[:, :], s3_t[:, :],
                        hfin[:, 1 + j * TC:1 + (j + 1) * TC],
                        start=True, stop=True, tile_position=(0, 0))
                    y_t = yp.tile([4 * NB, TC], f32, tag="y")
                    nc.scalar.activation(y_t[:], p3[:, :], AF.Sigmoid,
                                         bias=bdv_t[:, :])
                    nc.sync.dma_start(y_d.ap()[:, j * TC:(j + 1) * TC], y_t[:])

    nc.compile()
    return nc


def _host_consts(W, U, b, Wd, bd, T):
    """Pack the small parameter matrices into the stationary layouts."""
    W = np.asarray(W, np.float32)
    U = np.asarray(U, np.float32)
    b = np.asarray(b, np.float32)
    Wd = np.asarray(Wd, np.float32)
    bd = np.asarray(bd, np.float32)

    w2 = np.zeros((2 * F, 24), np.float32)
    w2[0:F, 0:12] = W
    w2[F:2 * F, 12:24] = W

    idb = np.zeros((L + 1, GATES * L), np.float32)
    bdu = np.zeros((L, GATES * L), np.float32)
    for gt in range(GATES):
        blk = idb[:, gt * L:(gt + 1) * L]
        blk[0:L, :] = np.eye(L, dtype=np.float32)
        for s in range(NS):
            for u in range(UNITS):
                blk[L, 3 * s + u] = b[3 * gt + u]
        ublk = bdu[:, gt * L:(gt + 1) * L]
        for s in range(NS):
            for up in range(UNITS):
                for u in range(UNITS):
                    ublk[3 * s + up, 3 * s + u] = U[up, 3 * gt + u]

    s3 = np.zeros((L, 4 * NB), np.float32)
    for bb in range(NB):
        for c in range(3):
            for u in range(UNITS):
                for d in range(4):
                    s3[9 * bb + 3 * c + u, 4 * bb + d] = Wd[3 * c + u, d]
    bdv = np.tile(bd, NB).reshape(4 * NB, 1).astype(np.float32)

    # phase-1 evacuation bias: stg row r = 32p + 12sl + 3gt + u -> b[3gt+u]
    bev = np.zeros((128, 1), np.float32)
    for p in range(4):
        for sl in range(2):
            for gt in range(GATES):
                for u in range(UNITS):
                    bev[32 * p + 12 * sl + 3 * gt + u, 0] = b[3 * gt + u]
    # sweep-1 bias per gate: lane 3s+u -> b[3gt+u]
    bg = np.zeros((L, GATES), np.float32)
    for gt in range(GATES):
        for s in range(NS):
            for u in range(UNITS):
                bg[3 * s + u, gt] = b[3 * gt + u]

    return {"w2": w2.astype(BF16), "idb": idb.astype(BF16),
            "bdu": bdu.astype(BF16), "s3": s3.astype(BF16),
            "bdv": bdv, "bev": bev, "bg": bg,
            "ones1": np.ones((1, GATES * T), BF16)}


def _host_xt(inputs, T):
    """[B, T, 192] -> per-core [NS, F, T] bf16 with s = 3*b_local + c."""
    B = inputs.shape[0]
    x = np.asarray(inputs, np.float32).reshape(B, T, 3, F)
    x = np.ascontiguousarray(np.transpose(x, (0, 2, 3, 1)))  # [B, c, F, T]
    x = x.astype(BF16)
    per_core = []
    for k in range(N_CORES):
        per_core.append(x[k * NB:(k + 1) * NB].reshape(NS, F, T))
    return per_core


def kernel(inputs, W, U, b, Wd, bd):
    from concourse.bass_utils import run_bass_kernel_spmd

    B, T, F3 = inputs.shape
    assert (B, T, F3) == (B_FULL, T_FULL, 192)

    key = (T, K_ITERS)
    if key not in _cache:
        _cache[key] = _build_module(T, K_ITERS, debug=False)
    nc = _cache[key]

    consts = _host_consts(W, U, b, Wd, bd, T)
    xts = _host_xt(inputs, T)
    in_maps = [dict(consts, xt=xts[k]) for k in range(N_CORES)]

    global _last_exec_ns
    res = run_bass_kernel_spmd(nc, in_maps, list(range(N_CORES)), trace=TRACE)
    if res.exec_time_ns is not None:
        _last_exec_ns = res.exec_time_ns
    ys = [res.results[k]["y"] for k in range(N_CORES)]  # [32, T] each

    out = np.empty((B, T, 4), np.float32)
    for k in range(N_CORES):
        blk = ys[k].reshape(NB, 4, T)          # [b, d, t]
        out[k * NB:(k + 1) * NB] = np.transpose(blk, (0, 2, 1))
    return out


# revision 12
# speedup vs baseline: 2.8256x; 1.6761x over previous
"""Trainium2 Bass kernel for nn_Mk1_91036126806096.

Shared-weight LSTM (3 units, all-sigmoid activations) over [192 folded
sequences x T=4096 x 64 features], followed by a 4-unit dense layer with
sigmoid.  Data-parallel over 8 NeuronCores (8 original batch elements,
i.e. 24 folded sequences, per core).

The sequential scan is replaced by a Picard fixed-point iteration: given
gate values the c-recurrence c_t = f_t*c_{t-1} + i_t*g_t is linear and
runs in one DVE tensor_tensor_scan instruction per 512-step chunk; the
gates are recomputed from the lagged h trajectory each sweep.  The
iteration contracts ~10x per sweep; K=3 sweeps give ~2.7e-3 rel error
end-to-end in bf16 (threshold 2e-2).

Layout: lane L = 3*s + u for folded sequence s = 3*b_local + c and unit
u; everything in phase 2 lives on lanes 0..71 with time (and the 4
gates, as 4 blocks) along the free dimension.

Numerics: x, weights, gates, c and h are bf16 (PE streams bf16 at full
rate; DVE gets 2x modes); PSUM accumulation is fp32.  The bias b rides
as a 73rd all-ones row of zpre against a b-row in the identity
stationary (sweeps 2+) / as a per-partition activation bias (phase 1
and sweep 1).
"""

import numpy as np
import ml_dtypes

BF16 = ml_dtypes.bfloat16

UNITS = 3
GATES = 4
B_FULL = 64
T_FULL = 4096
F = 64
N_CORES = 8
NB = 8                 # batch elements per core
NS = NB * 3            # folded sequences per core
L = NS * UNITS         # lanes = 72
TC = 512               # time chunk (one PSUM bank of fp32)
K_ITERS = 3            # Picard sweeps
NGROUP = 3             # seq-pair groups in phase 1 (4 pairs each)

_cache = {}
TRACE = False
_last_exec_ns = None


def _build_module(T, k_iters, debug):
    import concourse.bass as bass
    import concourse.tile as tile
    from concourse import bacc, mybir

    f32 = mybir.dt.float32
    bf16 = mybir.dt.bfloat16
    AF = mybir.ActivationFunctionType
    OP = mybir.AluOpType
    NCH = T // TC
    HT = T // 2

    nc = bacc.Bacc("TRN2", target_bir_lowering=False, debug=debug)

    # x, per core: [NS, F, T] bf16 with s = 3*b_local + c
    xt = nc.dram_tensor("xt", [NS, F, T], bf16, kind="ExternalInput")
    # W2: block-diag [128, 24]: rows 0:64 -> cols (seq a), rows 64:128
    # -> cols (seq b); column order within a seq-pair is 6*gt + 3*sl + u
    # so each gate owns 6 contiguous stg rows per pair.
    w2_d = nc.dram_tensor("w2", [2 * F, 24], bf16, kind="ExternalInput")
    # identity stationary [72, 72] (b is folded into zpre at evacuation)
    id_d = nc.dram_tensor("idm", [L, L], bf16, kind="ExternalInput")
    # block-diag U per gate [72, 4*72]
    bdu_d = nc.dram_tensor("bdu", [L, GATES * L], bf16, kind="ExternalInput")
    # dense stationary [72, 32] and bias [32, 1]
    s3_d = nc.dram_tensor("s3", [L, 4 * NB], bf16, kind="ExternalInput")
    bdv_d = nc.dram_tensor("bdv", [4 * NB, 1], f32, kind="ExternalInput")
    # phase-1 evacuation bias [128, 1] (b per stg row pattern)
    bev_d = nc.dram_tensor("bev", [128, 1], f32, kind="ExternalInput")
    y_d = nc.dram_tensor("y", [4 * NB, T], f32, kind="ExternalOutput")

    with tile.TileContext(nc) as tc:
        with tc.tile_pool(name="const", bufs=1) as cp, \
             tc.tile_pool(name="persist", bufs=1) as pp:
            w2_t = cp.tile([2 * F, 24], bf16, tag="w2")
            nc.sync.dma_start(w2_t[:], w2_d.ap())
            id_t = cp.tile([L, L], bf16, tag="idm")
            nc.sync.dma_start(id_t[:], id_d.ap())
            bdu_t = cp.tile([L, GATES * L], bf16, tag="bdu")
            nc.sync.dma_start(bdu_t[:], bdu_d.ap())
            s3_t = cp.tile([L, 4 * NB], bf16, tag="s3")
            nc.sync.dma_start(s3_t[:], s3_d.ap())
            bdv_t = cp.tile([4 * NB, 1], f32, tag="bdv")
            nc.sync.dma_start(bdv_t[:], bdv_d.ap())
            bev_t = cp.tile([128, 1], f32, tag="bev")
            nc.sync.dma_start(bev_t[:], bev_d.ap())

            # zpre: [72, GATES*T] bf16, gate-major; z = x@W + b
            zpre = pp.tile([L, GATES * T], bf16, tag="zpre")
            hA = pp.tile([L, 1 + T], bf16, tag="hA")
            hB = pp.tile([L, 1 + T], bf16, tag="hB")
            nc.vector.memset(hA[:, 0:1], 0.0)
            nc.vector.memset(hB[:, 0:1], 0.0)

            # ---------------- Phase 1: zpre = x @ W + b ----------------
            # 4 seq-pairs per PSUM tile via column tiling (out partitions
            # 32p..32p+24, gate-major rows within a pair); DVE evacuates
            # with the per-partition bias b; scatter DMAs (one per pair,
            # gate) land 6-row blocks into zpre's gate-major layout.
            with tc.tile_pool(name="xp", bufs=3) as xp, \
                 tc.tile_pool(name="stgp", bufs=2) as stgp, \
                 tc.tile_pool(name="ps1", bufs=2, space="PSUM") as ps1p:
                it = 0
                for half in range(2):
                    for g in range(NGROUP):
                        xts = []
                        for p in range(4):
                            xtl = xp.tile([128, HT], bf16, tag=f"x{p}")
                            pr = 8 * g + 2 * p
                            nc.sync.dma_start(
                                xtl[:], xt.ap()[pr:pr + 2, :,
                                                half * HT:(half + 1) * HT])
                            xts.append(xtl)
                        stg = stgp.tile([128, HT], bf16, tag="stg")
                        for j in range(HT // TC):
                            pt = ps1p.tile([128, TC], f32, tag="p1")
                            for p in range(4):
                                nc.tensor.matmul(
                                    pt[32 * p:32 * p + 24, :],
                                    w2_t[:, :],
                                    xts[p][:, j * TC:(j + 1) * TC],
                                    start=True, stop=True,
                                    tile_position=(0, 32 * p))
                            nc.vector.tensor_scalar(
                                out=stg[:, j * TC:(j + 1) * TC], in0=pt[:, :],
                                scalar1=bev_t[:, :], scalar2=None, op0=OP.add)
                        # scatter: per (pair, gate) one [6, HT] DMA
                        for p in range(4):
                            for gt in range(GATES):
                                eng = nc.sync if it % 2 == 0 else nc.gpsimd
                                it += 1
                                eng.dma_start(
                                    zpre[24 * g + 6 * p:24 * g + 6 * p + 6,
                                         gt * T + half * HT:
                                         gt * T + (half + 1) * HT],
                                    stg[32 * p + 6 * gt:32 * p + 6 * gt + 6, :])

            # ---------------- Phase 2: Picard sweeps ----------------
            hbufs = [hA, hB]
            with tc.tile_pool(name="sp", bufs=3) as sp, \
                 tc.tile_pool(name="igp", bufs=2) as igp, \
                 tc.tile_pool(name="scp", bufs=2) as scp, \
                 tc.tile_pool(name="cpool", bufs=3) as cpl, \
                 tc.tile_pool(name="zps", bufs=2, space="PSUM") as zpsp:
                for k in range(k_iters):
                    hold = hbufs[k % 2]
                    hnew = hbufs[(k + 1) % 2]
                    c_prev = None
                    for j in range(NCH):
                        s_t = sp.tile([L, GATES * TC], bf16, tag="s")
                        if k == 0:
                            # gates straight from zpre (h == 0): one ACT
                            # over a 4-gate strided view
                            src = zpre[:].rearrange(
                                "l (g t) -> l g t", g=GATES)[
                                :, :, j * TC:(j + 1) * TC]
                            dst = s_t[:].rearrange(
                                "l (g t) -> l g t", g=GATES)
                            nc.scalar.activation(dst, src, AF.Sigmoid)
                        else:
                            zps = zpsp.tile([L, GATES * TC], f32, tag="zps")
                            for gt in range(GATES):
                                nc.tensor.matmul(
                                    zps[:, gt * TC:(gt + 1) * TC],
                                    id_t[:, :],
                                    zpre[:, gt * T + j * TC:
                                         gt * T + (j + 1) * TC],
                                    start=True, stop=False,
                                    tile_position=(0, 0))
                                nc.tensor.matmul(
                                    zps[:, gt * TC:(gt + 1) * TC],
                                    bdu_t[:, gt * L:(gt + 1) * L],
                                    hold[:, j * TC:(j + 1) * TC],
                                    start=False, stop=True,
                                    tile_position=(0, 0))
                            nc.scalar.activation(s_t[:], zps[:, :], AF.Sigmoid)
                        ig = igp.tile([L, TC], bf16, tag="ig")
                        nc.vector.tensor_tensor(
                            out=ig[:], in0=s_t[:, 0:TC],
                            in1=s_t[:, 2 * TC:3 * TC], op=OP.mult)
                        c_t = cpl.tile([L, TC], bf16, tag="c")
                        init = 0.0 if j == 0 else c_prev[:, TC - 1:TC]
                        nc.vector.tensor_tensor_scan(
                            out=c_t[:], data0=s_t[:, TC:2 * TC], data1=ig[:],
                            initial=init, op0=OP.mult, op1=OP.add)
                        c_prev = c_t
                        sc_t = scp.tile([L, TC], bf16, tag="sc")
                        nc.scalar.activation(sc_t[:], c_t[:], AF.Sigmoid)
                        nc.vector.tensor_tensor(
                            out=hnew[:, 1 + j * TC:1 + (j + 1) * TC],
                            in0=s_t[:, 3 * TC:4 * TC], in1=sc_t[:], op=OP.mult)

            # ---------------- Phase 3: dense + sigmoid -------
            hfin = hbufs[k_iters % 2]
            with tc.tile_pool(name="yp", bufs=2) as yp, \
                 tc.tile_pool(name="ps3", bufs=2, space="PSUM") as ps3p:
                for j in range(NCH):
                    p3 = ps3p.tile([4 * NB, TC], f32, tag="p3")
                    nc.tensor.matmul(
                        p3[:, :], s3_t[:, :],
                        hfin[:, 1 + j * TC:1 + (j + 1) * TC],
                        start=True, stop=True, tile_position=(0, 0))
                    y_t = yp.tile([4 * NB, TC], f32, tag="y")
                    nc.scalar.activation(y_t[:], p3[:, :], AF.Sigmoid,
                                         bias=bdv_t[:, :])
                    nc.sync.dma_start(y_d.ap()[:, j * TC:(j + 1) * TC], y_t[:])

    nc.compile()
    return nc


def _host_consts(W, U, b, Wd, bd, T):
    """Pack the small parameter matrices into the stationary layouts."""
    W = np.asarray(W, np.float32)
    U = np.asarray(U, np.float32)
    b = np.asarray(b, np.float32)
    Wd = np.asarray(Wd, np.float32)
    bd = np.asarray(bd, np.float32)

    # W2 column (within a seq-pair) = 6*gt + 3*sl + u; sl = seq in pair
    w2 = np.zeros((2 * F, 24), np.float32)
    for gt in range(GATES):
        for sl in range(2):
            for u in range(UNITS):
                w2[sl * F:(sl + 1) * F, 6 * gt + 3 * sl + u] = W[:, 3 * gt + u]

    idm = np.eye(L, dtype=np.float32)
    bdu = np.zeros((L, GATES * L), np.float32)
    for gt in range(GATES):
        ublk = bdu[:, gt * L:(gt + 1) * L]
        for s in range(NS):
            for up in range(UNITS):
                for u in range(UNITS):
                    ublk[3 * s + up, 3 * s + u] = U[up, 3 * gt + u]

    s3 = np.zeros((L, 4 * NB), np.float32)
    for bb in range(NB):
        for c in range(3):
            for u in range(UNITS):
                for d in range(4):
                    s3[9 * bb + 3 * c + u, 4 * bb + d] = Wd[3 * c + u, d]
    bdv = np.tile(bd, NB).reshape(4 * NB, 1).astype(np.float32)

    # phase-1 evacuation bias: stg row r = 32p + 6gt + 3sl + u -> b[3gt+u]
    bev = np.zeros((128, 1), np.float32)
    for p in range(4):
        for gt in range(GATES):
            for sl in range(2):
                for u in range(UNITS):
                    bev[32 * p + 6 * gt + 3 * sl + u, 0] = b[3 * gt + u]

    return {"w2": w2.astype(BF16), "idm": idm.astype(BF16),
            "bdu": bdu.astype(BF16), "s3": s3.astype(BF16),
            "bdv": bdv, "bev": bev}


def _host_xt(inputs, T):
    """[B, T, 192] -> per-core [NS, F, T] bf16 with s = 3*b_local + c."""
    B = inputs.shape[0]
    x = np.asarray(inputs, np.float32).reshape(B, T, 3, F)
    x = np.ascontiguousarray(np.transpose(x, (0, 2, 3, 1)))  # [B, c, F, T]
    x = x.astype(BF16)
    per_core = []
    for k in range(N_CORES):
        per_core.append(x[k * NB:(k + 1) * NB].reshape(NS, F, T))
    return per_core


def kernel(inputs, W, U, b, Wd, bd):
    from concourse.bass_utils import run_bass_kernel_spmd

    B, T, F3 = inputs.shape
    assert (B, T, F3) == (B_FULL, T_FULL, 192)

    key = (T, K_ITERS)
    if key not in _cache:
        _cache[key] = _build_module(T, K_ITERS, debug=False)
    nc = _cache[key]

    consts = _host_consts(W, U, b, Wd, bd, T)
    xts = _host_xt(inputs, T)
    in_maps = [dict(consts, xt=xts[k]) for k in range(N_CORES)]

    global _last_exec_ns
    res = run_bass_kernel_spmd(nc, in_maps, list(range(N_CORES)), trace=TRACE)
    if res.exec_time_ns is not None:
        _last_exec_ns = res.exec_time_ns
    ys = [res.results[k]["y"] for k in range(N_CORES)]  # [32, T] each

    out = np.empty((B, T, 4), np.float32)
    for k in range(N_CORES):
        blk = ys[k].reshape(NB, 4, T)          # [b, d, t]
        out[k * NB:(k + 1) * NB] = np.transpose(blk, (0, 2, 1))
    return out


# revision 15
# speedup vs baseline: 2.8885x; 1.0222x over previous
"""Trainium2 Bass kernel for nn_Mk1_91036126806096.

Shared-weight LSTM (3 units, all-sigmoid activations) over [192 folded
sequences x T=4096 x 64 features], followed by a 4-unit dense layer with
sigmoid.  Data-parallel over 8 NeuronCores (8 original batch elements,
i.e. 24 folded sequences, per core).

The sequential scan is replaced by a Picard fixed-point iteration: given
gate values the c-recurrence c_t = f_t*c_{t-1} + i_t*g_t is linear and
runs in one DVE tensor_tensor_scan instruction per 512-step chunk; the
gates are recomputed from the lagged h trajectory each sweep.  The
iteration contracts ~10x per sweep; K=3 sweeps give ~2.7e-3 rel error
end-to-end in bf16 (threshold 2e-2).

Layout: lane L = 3*s + u for folded sequence s = 3*b_local + c and unit
u; everything in phase 2 lives on lanes 0..71 with time (and the 4
gates, as 4 blocks) along the free dimension.

Numerics: x, weights, gates, c and h are bf16 (PE streams bf16 at full
rate; DVE gets 2x modes); PSUM accumulation is fp32.  The bias b rides
as a 73rd all-ones row of zpre against a b-row in the identity
stationary (sweeps 2+) / as a per-partition activation bias (phase 1
and sweep 1).
"""

import numpy as np
import ml_dtypes

BF16 = ml_dtypes.bfloat16

UNITS = 3
GATES = 4
B_FULL = 64
T_FULL = 4096
F = 64
N_CORES = 8
NB = 8                 # batch elements per core
NS = NB * 3            # folded sequences per core
L = NS * UNITS         # lanes = 72
TC = 512               # time chunk (one PSUM bank of fp32)
K_ITERS = 3            # Picard sweeps
NGROUP = 3             # seq-pair groups in phase 1 (4 pairs each)

_cache = {}
TRACE = False
_last_exec_ns = None


def _build_module(T, k_iters, debug):
    import concourse.bass as bass
    import concourse.tile as tile
    from concourse import bacc, mybir

    f32 = mybir.dt.float32
    bf16 = mybir.dt.bfloat16
    AF = mybir.ActivationFunctionType
    OP = mybir.AluOpType
    NCH = T // TC
    HT = T // 2

    nc = bacc.Bacc("TRN2", target_bir_lowering=False, debug=debug)

    # x, per core: [NS, F, T] bf16 with s = 3*b_local + c
    xt = nc.dram_tensor("xt", [NS, F, T], bf16, kind="ExternalInput")
    # W2: block-diag [128, 24]: rows 0:64 -> cols (seq a), rows 64:128
    # -> cols (seq b); column order within a seq-pair is 6*gt + 3*sl + u
    # so each gate owns 6 contiguous stg rows per pair.
    w2_d = nc.dram_tensor("w2", [2 * F, 24], bf16, kind="ExternalInput")
    # identity stationary [72, 72] (b is folded into zpre at evacuation)
    id_d = nc.dram_tensor("idm", [L, L], bf16, kind="ExternalInput")
    # block-diag U per gate [72, 4*72]
    bdu_d = nc.dram_tensor("bdu", [L, GATES * L], bf16, kind="ExternalInput")
    # dense stationary [72, 32] and bias [32, 1]
    s3_d = nc.dram_tensor("s3", [L, 4 * NB], bf16, kind="ExternalInput")
    bdv_d = nc.dram_tensor("bdv", [4 * NB, 1], f32, kind="ExternalInput")
    # phase-1 evacuation bias [128, 1] (b per stg row pattern)
    bev_d = nc.dram_tensor("bev", [128, 1], f32, kind="ExternalInput")
    y_d = nc.dram_tensor("y", [4 * NB, T], f32, kind="ExternalOutput")

    with tile.TileContext(nc) as tc:
        with tc.tile_pool(name="const", bufs=1) as cp, \
             tc.tile_pool(name="persist", bufs=1) as pp:
            w2_t = cp.tile([2 * F, 24], bf16, tag="w2")
            nc.sync.dma_start(w2_t[:], w2_d.ap())
            id_t = cp.tile([L, L], bf16, tag="idm")
            nc.sync.dma_start(id_t[:], id_d.ap())
            bdu_t = cp.tile([L, GATES * L], bf16, tag="bdu")
            nc.sync.dma_start(bdu_t[:], bdu_d.ap())
            s3_t = cp.tile([L, 4 * NB], bf16, tag="s3")
            nc.sync.dma_start(s3_t[:], s3_d.ap())
            bdv_t = cp.tile([4 * NB, 1], f32, tag="bdv")
            nc.sync.dma_start(bdv_t[:], bdv_d.ap())
            bev_t = cp.tile([128, 1], f32, tag="bev")
            nc.sync.dma_start(bev_t[:], bev_d.ap())

            # zpre: [72, GATES*T] bf16, gate-major; z = x@W + b
            zpre = pp.tile([L, GATES * T], bf16, tag="zpre")
            hA = pp.tile([L, 1 + T], bf16, tag="hA")
            hB = pp.tile([L, 1 + T], bf16, tag="hB")
            nc.vector.memset(hA[:, 0:1], 0.0)
            nc.vector.memset(hB[:, 0:1], 0.0)

            # ---------------- Phase 1: zpre = x @ W + b ----------------
            # 4 seq-pairs per PSUM tile via column tiling (out partitions
            # 32p..32p+24, gate-major rows within a pair); DVE evacuates
            # with the per-partition bias b; scatter DMAs (one per pair,
            # gate) land 6-row blocks into zpre's gate-major layout.
            with tc.tile_pool(name="xp", bufs=1) as xp, \
                 tc.tile_pool(name="stgp", bufs=2) as stgp, \
                 tc.tile_pool(name="ps1", bufs=2, space="PSUM") as ps1p:
                # issue every x load up front on the sync ring so scatter
                # sem-waits never block the x stream
                xtiles = {}
                for half in range(2):
                    for g in range(NGROUP):
                        for p in range(4):
                            xtl = xp.tile([128, HT], bf16, tag=f"x{half}{g}{p}")
                            pr = 8 * g + 2 * p
                            nc.sync.dma_start(
                                xtl[:], xt.ap()[pr:pr + 2, :,
                                                half * HT:(half + 1) * HT])
                            xtiles[(half, g, p)] = xtl
                it = 0
                for half in range(2):
                    for g in range(NGROUP):
                        stg = stgp.tile([128, HT], bf16, tag="stg")
                        for j in range(HT // TC):
                            pt = ps1p.tile([128, TC], f32, tag="p1")
                            for p in range(4):
                                nc.tensor.matmul(
                                    pt[32 * p:32 * p + 24, :],
                                    w2_t[:, :],
                                    xtiles[(half, g, p)][:, j * TC:(j + 1) * TC],
                                    start=True, stop=True,
                                    tile_position=(0, 32 * p))
                            nc.vector.tensor_scalar(
                                out=stg[:, j * TC:(j + 1) * TC], in0=pt[:, :],
                                scalar1=bev_t[:, :], scalar2=None, op0=OP.add)
                        # scatter: per (pair, gate) one [6, HT] DMA
                        for p in range(4):
                            for gt in range(GATES):
                                eng = nc.sync if it % 3 == 0 else nc.gpsimd
                                it += 1
                                eng.dma_start(
                                    zpre[24 * g + 6 * p:24 * g + 6 * p + 6,
                                         gt * T + half * HT:
                                         gt * T + (half + 1) * HT],
                                    stg[32 * p + 6 * gt:32 * p + 6 * gt + 6, :])

            # ---------------- Phase 2: Picard sweeps ----------------
            hbufs = [hA, hB]
            with tc.tile_pool(name="sp", bufs=3) as sp, \
                 tc.tile_pool(name="igp", bufs=2) as igp, \
                 tc.tile_pool(name="scp", bufs=2) as scp, \
                 tc.tile_pool(name="cpool", bufs=3) as cpl, \
                 tc.tile_pool(name="zps", bufs=2, space="PSUM") as zpsp:
                for k in range(k_iters):
                    hold = hbufs[k % 2]
                    hnew = hbufs[(k + 1) % 2]
                    c_prev = None
                    for j in range(NCH):
                        s_t = sp.tile([L, GATES * TC], bf16, tag="s")
                        if k == 0:
                            # gates straight from zpre (h == 0): one ACT
                            # over a 4-gate strided view
                            src = zpre[:].rearrange(
                                "l (g t) -> l g t", g=GATES)[
                                :, :, j * TC:(j + 1) * TC]
                            dst = s_t[:].rearrange(
                                "l (g t) -> l g t", g=GATES)
                            nc.scalar.activation(dst, src, AF.Sigmoid)
                        else:
                            zps = zpsp.tile([L, GATES * TC], f32, tag="zps")
                            # all 4 identity matmuls first (one stationary
                            # load), then the 4 U-feedback accumulations
                            for gt in range(GATES):
                                nc.tensor.matmul(
                                    zps[:, gt * TC:(gt + 1) * TC],
                                    id_t[:, :],
                                    zpre[:, gt * T + j * TC:
                                         gt * T + (j + 1) * TC],
                                    start=True, stop=False,
                                    tile_position=(0, 0))
                            for gt in range(GATES):
                                nc.tensor.matmul(
                                    zps[:, gt * TC:(gt + 1) * TC],
                                    bdu_t[:, gt * L:(gt + 1) * L],
                                    hold[:, j * TC:(j + 1) * TC],
                                    start=False, stop=True,
                                    tile_position=(0, 0))
                            nc.scalar.activation(s_t[:], zps[:, :], AF.Sigmoid)
                        ig = igp.tile([L, TC], bf16, tag="ig")
                        nc.vector.tensor_tensor(
                            out=ig[:], in0=s_t[:, 0:TC],
                            in1=s_t[:, 2 * TC:3 * TC], op=OP.mult)
                        c_t = cpl.tile([L, TC], bf16, tag="c")
                        init = 0.0 if j == 0 else c_prev[:, TC - 1:TC]
                        nc.vector.tensor_tensor_scan(
                            out=c_t[:], data0=s_t[:, TC:2 * TC], data1=ig[:],
                            initial=init, op0=OP.mult, op1=OP.add)
                        c_prev = c_t
                        sc_t = scp.tile([L, TC], bf16, tag="sc")
                        nc.scalar.activation(sc_t[:], c_t[:], AF.Sigmoid)
                        nc.vector.tensor_tensor(
                            out=hnew[:, 1 + j * TC:1 + (j + 1) * TC],
                            in0=s_t[:, 3 * TC:4 * TC], in1=sc_t[:], op=OP.mult)

            # ---------------- Phase 3: dense + sigmoid -------
            hfin = hbufs[k_iters % 2]
            with tc.tile_pool(name="yp", bufs=1) as yp, \
                 tc.tile_pool(name="ps3", bufs=2, space="PSUM") as ps3p:
                y_t = yp.tile([4 * NB, T], f32, tag="y")
                for j in range(NCH):
                    p3 = ps3p.tile([4 * NB, TC], f32, tag="p3")
                    nc.tensor.matmul(
                        p3[:, :], s3_t[:, :],
                        hfin[:, 1 + j * TC:1 + (j + 1) * TC],
                        start=True, stop=True, tile_position=(0, 0))
                    nc.scalar.activation(y_t[:, j * TC:(j + 1) * TC], p3[:, :],
                                         AF.Sigmoid, bias=bdv_t[:, :])
                nc.sync.dma_start(y_d.ap(), y_t[:])

    nc.compile()
    return nc


def _host_consts(W, U, b, Wd, bd, T):
    """Pack the small parameter matrices into the stationary layouts."""
    W = np.asarray(W, np.float32)
    U = np.asarray(U, np.float32)
    b = np.asarray(b, np.float32)
    Wd = np.asarray(Wd, np.float32)
    bd = np.asarray(bd, np.float32)

    # W2 column (within a seq-pair) = 6*gt + 3*sl + u; sl = seq in pair
    w2 = np.zeros((2 * F, 24), np.float32)
    for gt in range(GATES):
        for sl in range(2):
            for u in range(UNITS):
                w2[sl * F:(sl + 1) * F, 6 * gt + 3 * sl + u] = W[:, 3 * gt + u]

    idm = np.eye(L, dtype=np.float32)
    bdu = np.zeros((L, GATES * L), np.float32)
    for gt in range(GATES):
        ublk = bdu[:, gt * L:(gt + 1) * L]
        for s in range(NS):
            for up in range(UNITS):
                for u in range(UNITS):
                    ublk[3 * s + up, 3 * s + u] = U[up, 3 * gt + u]

    s3 = np.zeros((L, 4 * NB), np.float32)
    for bb in range(NB):
        for c in range(3):
            for u in range(UNITS):
                for d in range(4):
                    s3[9 * bb + 3 * c + u, 4 * bb + d] = Wd[3 * c + u, d]
    bdv = np.tile(bd, NB).reshape(4 * NB, 1).astype(np.float32)

    # phase-1 evacuation bias: stg row r = 32p + 6gt + 3sl + u -> b[3gt+u]
    bev = np.zeros((128, 1), np.float32)
    for p in range(4):
        for gt in range(GATES):
            for sl in range(2):
                for u in range(UNITS):
                    bev[32 * p + 6 * gt + 3 * sl + u, 0] = b[3 * gt + u]

    return {"w2": w2.astype(BF16), "idm": idm.astype(BF16),
            "bdu": bdu.astype(BF16), "s3": s3.astype(BF16),
            "bdv": bdv, "bev": bev}


def _host_xt(inputs, T):
    """[B, T, 192] -> per-core [NS, F, T] bf16 with s = 3*b_local + c."""
    B = inputs.shape[0]
    x = np.asarray(inputs, np.float32).reshape(B, T, 3, F)
    x = np.ascontiguousarray(np.transpose(x, (0, 2, 3, 1)))  # [B, c, F, T]
    x = x.astype(BF16)
    per_core = []
    for k in range(N_CORES):
        per_core.append(x[k * NB:(k + 1) * NB].reshape(NS, F, T))
    return per_core


def kernel(inputs, W, U, b, Wd, bd):
    from concourse.bass_utils import run_bass_kernel_spmd

    B, T, F3 = inputs.shape
    assert (B, T, F3) == (B_FULL, T_FULL, 192)

    key = (T, K_ITERS)
    if key not in _cache:
        _cache[key] = _build_module(T, K_ITERS, debug=False)
    nc = _cache[key]

    consts = _host_consts(W, U, b, Wd, bd, T)
    xts = _host_xt(inputs, T)
    in_maps = [dict(consts, xt=xts[k]) for k in range(N_CORES)]

    global _last_exec_ns
    res = run_bass_kernel_spmd(nc, in_maps, list(range(N_CORES)), trace=TRACE)
    if res.exec_time_ns is not None:
        _last_exec_ns = res.exec_time_ns
    ys = [res.results[k]["y"] for k in range(N_CORES)]  # [32, T] each

    out = np.empty((B, T, 4), np.float32)
    for k in range(N_CORES):
        blk = ys[k].reshape(NB, 4, T)          # [b, d, t]
        out[k * NB:(k + 1) * NB] = np.transpose(blk, (0, 2, 1))
    return out


# revision 18
# speedup vs baseline: 3.1597x; 1.0939x over previous
"""Trainium2 Bass kernel for nn_Mk1_91036126806096.

Shared-weight LSTM (3 units, all-sigmoid activations) over [192 folded
sequences x T=4096 x 64 features], followed by a 4-unit dense layer with
sigmoid.  Data-parallel over 8 NeuronCores (8 original batch elements,
i.e. 24 folded sequences, per core).

The sequential scan is replaced by a Picard fixed-point iteration: given
gate values the c-recurrence c_t = f_t*c_{t-1} + i_t*g_t is linear and
runs in one DVE tensor_tensor_scan instruction per 512-step chunk; the
gates are recomputed from the lagged h trajectory each sweep.  The
iteration contracts ~10x per sweep; K=3 sweeps give ~2.7e-3 rel error
end-to-end in bf16 (threshold 2e-2).

Layout: lane L = 3*s + u for folded sequence s = 3*b_local + c and unit
u; everything in phase 2 lives on lanes 0..71 with time (and the 4
gates, as 4 blocks) along the free dimension.

Numerics: x, weights, gates, c and h are bf16 (PE streams bf16 at full
rate; DVE gets 2x modes); PSUM accumulation is fp32.  The bias b rides
as a 73rd all-ones row of zpre against a b-row in the identity
stationary (sweeps 2+) / as a per-partition activation bias (phase 1
and sweep 1).
"""

import numpy as np
import ml_dtypes

BF16 = ml_dtypes.bfloat16

UNITS = 3
GATES = 4
B_FULL = 64
T_FULL = 4096
F = 64
N_CORES = 8
NB = 8                 # batch elements per core
NS = NB * 3            # folded sequences per core
L = NS * UNITS         # lanes = 72
TC = 512               # time chunk (one PSUM bank of fp32)
K_ITERS = 3            # Picard sweeps
NGROUP = 3             # seq-pair groups in phase 1 (4 pairs each)

_cache = {}
TRACE = False
_last_exec_ns = None


def _build_module(T, k_iters, debug):
    import concourse.bass as bass
    import concourse.tile as tile
    from concourse import bacc, mybir

    f32 = mybir.dt.float32
    bf16 = mybir.dt.bfloat16
    AF = mybir.ActivationFunctionType
    OP = mybir.AluOpType
    NCH = T // TC
    HT = T // 2

    nc = bacc.Bacc("TRN2", target_bir_lowering=False, debug=debug)

    # x, per core: [NS, F, T] bf16 with s = 3*b_local + c
    xt = nc.dram_tensor("xt", [NS, F, T], bf16, kind="ExternalInput")
    # W2: block-diag [128, 24]: rows 0:64 -> cols (seq a), rows 64:128
    # -> cols (seq b); column order within a seq-pair is 6*gt + 3*sl + u
    # so each gate owns 6 contiguous stg rows per pair.
    w2_d = nc.dram_tensor("w2", [2 * F, 24], bf16, kind="ExternalInput")
    # identity stationary [72, 72] (b is folded into zpre at evacuation)
    id_d = nc.dram_tensor("idm", [L, L], bf16, kind="ExternalInput")
    # block-diag U per gate [72, 4*72]
    bdu_d = nc.dram_tensor("bdu", [L, GATES * L], bf16, kind="ExternalInput")
    # dense stationary [72, 32] and bias [32, 1]
    s3_d = nc.dram_tensor("s3", [L, 4 * NB], bf16, kind="ExternalInput")
    bdv_d = nc.dram_tensor("bdv", [4 * NB, 1], f32, kind="ExternalInput")
    # phase-1 evacuation bias [128, 1] (b per stg row pattern)
    bev_d = nc.dram_tensor("bev", [128, 1], f32, kind="ExternalInput")
    y_d = nc.dram_tensor("y", [4 * NB, T], f32, kind="ExternalOutput")
    # DRAM bounce buffer for the stg -> zpre gate scatter: one upload per
    # (half, group), one big strided download per half
    zs_d = nc.dram_tensor("zs", [2 * NGROUP, 128, HT], bf16, kind="Internal")

    with tile.TileContext(nc) as tc:
        with tc.tile_pool(name="const", bufs=1) as cp, \
             tc.tile_pool(name="persist", bufs=1) as pp:
            w2_t = cp.tile([2 * F, 24], bf16, tag="w2")
            nc.sync.dma_start(w2_t[:], w2_d.ap())
            id_t = cp.tile([L, L], bf16, tag="idm")
            nc.sync.dma_start(id_t[:], id_d.ap())
            bdu_t = cp.tile([L, GATES * L], bf16, tag="bdu")
            nc.sync.dma_start(bdu_t[:], bdu_d.ap())
            s3_t = cp.tile([L, 4 * NB], bf16, tag="s3")
            nc.sync.dma_start(s3_t[:], s3_d.ap())
            bdv_t = cp.tile([4 * NB, 1], f32, tag="bdv")
            nc.sync.dma_start(bdv_t[:], bdv_d.ap())
            bev_t = cp.tile([128, 1], f32, tag="bev")
            nc.sync.dma_start(bev_t[:], bev_d.ap())

            # zpre: [72, GATES*T] bf16, gate-major; z = x@W + b
            zpre = pp.tile([L, GATES * T], bf16, tag="zpre")
            hA = pp.tile([L, 1 + T], bf16, tag="hA")
            hB = pp.tile([L, 1 + T], bf16, tag="hB")
            nc.vector.memset(hA[:, 0:1], 0.0)
            nc.vector.memset(hB[:, 0:1], 0.0)

            # ---------------- Phase 1: zpre = x @ W + b ----------------
            # 4 seq-pairs per PSUM tile via column tiling (out partitions
            # 32p..32p+24, gate-major rows within a pair); DVE evacuates
            # with the per-partition bias b; scatter DMAs (one per pair,
            # gate) land 6-row blocks into zpre's gate-major layout.
            with tc.tile_pool(name="xp", bufs=1) as xp, \
                 tc.tile_pool(name="stgp", bufs=2) as stgp, \
                 tc.tile_pool(name="ps1", bufs=2, space="PSUM") as ps1p:
                # issue every x load up front on the sync ring so scatter
                # sem-waits never block the x stream
                xtiles = {}
                for half in range(2):
                    for g in range(NGROUP):
                        for p in range(4):
                            xtl = xp.tile([128, HT], bf16, tag=f"x{half}{g}{p}")
                            pr = 8 * g + 2 * p
                            nc.sync.dma_start(
                                xtl[:], xt.ap()[pr:pr + 2, :,
                                                half * HT:(half + 1) * HT])
                            xtiles[(half, g, p)] = xtl
                for half in range(2):
                    for g in range(NGROUP):
                        stg = stgp.tile([128, HT], bf16, tag="stg")
                        for j in range(HT // TC):
                            pt = ps1p.tile([128, TC], f32, tag="p1")
                            for p in range(4):
                                nc.tensor.matmul(
                                    pt[32 * p:32 * p + 24, :],
                                    w2_t[:, :],
                                    xtiles[(half, g, p)][:, j * TC:(j + 1) * TC],
                                    start=True, stop=True,
                                    tile_position=(0, 32 * p))
                            nc.vector.tensor_scalar(
                                out=stg[:, j * TC:(j + 1) * TC], in0=pt[:, :],
                                scalar1=bev_t[:, :], scalar2=None, op0=OP.add)
                        # upload the group's stg block to the DRAM bounce
                        nc.scalar.dma_start(
                            zs_d.ap()[3 * half + g:3 * half + g + 1], stg[:])
                    # strided downloads land the half's gate-major zpre:
                    # zs row 32p + 6gt + rr  ->  zpre lane 24g + 6p + rr
                    # (one DMA per gate; the AP balancer caps at 3 dims)
                    for gt in range(GATES):
                        src = zs_d.ap()[3 * half:3 * half + 3]
                        src = src.rearrange("g (p q) t -> (g p) q t", p=4)
                        src = src[:, 6 * gt:6 * gt + 6, :]
                        nc.sync.dma_start(
                            zpre[:, gt * T + half * HT:
                                 gt * T + (half + 1) * HT], src)

            # ---------------- Phase 2: Picard sweeps ----------------
            hbufs = [hA, hB]
            with tc.tile_pool(name="sp", bufs=3) as sp, \
                 tc.tile_pool(name="igp", bufs=2) as igp, \
                 tc.tile_pool(name="scp", bufs=2) as scp, \
                 tc.tile_pool(name="cpool", bufs=3) as cpl, \
                 tc.tile_pool(name="zps", bufs=2, space="PSUM") as zpsp:
                for k in range(k_iters):
                    hold = hbufs[k % 2]
                    hnew = hbufs[(k + 1) % 2]
                    c_prev = None
                    for j in range(NCH):
                        s_t = sp.tile([L, GATES * TC], bf16, tag="s")
                        if k == 0:
                            # gates straight from zpre (h == 0): one ACT
                            # over a 4-gate strided view
                            src = zpre[:].rearrange(
                                "l (g t) -> l g t", g=GATES)[
                                :, :, j * TC:(j + 1) * TC]
                            dst = s_t[:].rearrange(
                                "l (g t) -> l g t", g=GATES)
                            nc.scalar.activation(dst, src, AF.Sigmoid)
                        else:
                            zps = zpsp.tile([L, GATES * TC], f32, tag="zps")
                            # all 4 identity matmuls first (one stationary
                            # load), then the 4 U-feedback accumulations
                            for gt in range(GATES):
                                nc.tensor.matmul(
                                    zps[:, gt * TC:(gt + 1) * TC],
                                    id_t[:, :],
                                    zpre[:, gt * T + j * TC:
                                         gt * T + (j + 1) * TC],
                                    start=True, stop=False,
                                    tile_position=(0, 0))
                            for gt in range(GATES):
                                nc.tensor.matmul(
                                    zps[:, gt * TC:(gt + 1) * TC],
                                    bdu_t[:, gt * L:(gt + 1) * L],
                                    hold[:, j * TC:(j + 1) * TC],
                                    start=False, stop=True,
                                    tile_position=(0, 0))
                            nc.scalar.activation(s_t[:], zps[:, :], AF.Sigmoid)
                        ig = igp.tile([L, TC], bf16, tag="ig")
                        nc.vector.tensor_tensor(
                            out=ig[:], in0=s_t[:, 0:TC],
                            in1=s_t[:, 2 * TC:3 * TC], op=OP.mult)
                        c_t = cpl.tile([L, TC], bf16, tag="c")
                        init = 0.0 if j == 0 else c_prev[:, TC - 1:TC]
                        nc.vector.tensor_tensor_scan(
                            out=c_t[:], data0=s_t[:, TC:2 * TC], data1=ig[:],
                            initial=init, op0=OP.mult, op1=OP.add)
                        c_prev = c_t
                        sc_t = scp.tile([L, TC], bf16, tag="sc")
                        nc.scalar.activation(sc_t[:], c_t[:], AF.Sigmoid)
                        nc.vector.tensor_tensor(
                            out=hnew[:, 1 + j * TC:1 + (j + 1) * TC],
                            in0=s_t[:, 3 * TC:4 * TC], in1=sc_t[:], op=OP.mult)

            # ---------------- Phase 3: dense + sigmoid -------
            hfin = hbufs[k_iters % 2]
            with tc.tile_pool(name="yp", bufs=1) as yp, \
                 tc.tile_pool(name="ps3", bufs=2, space="PSUM") as ps3p:
                y_t = yp.tile([4 * NB, T], f32, tag="y")
                for j in range(NCH):
                    p3 = ps3p.tile([4 * NB, TC], f32, tag="p3")
                    nc.tensor.matmul(
                        p3[:, :], s3_t[:, :],
                        hfin[:, 1 + j * TC:1 + (j + 1) * TC],
                        start=True, stop=True, tile_position=(0, 0))
                    nc.scalar.activation(y_t[:, j * TC:(j + 1) * TC], p3[:, :],
                                         AF.Sigmoid, bias=bdv_t[:, :])
                nc.sync.dma_start(y_d.ap(), y_t[:])

    nc.compile()
    return nc


def _host_consts(W, U, b, Wd, bd, T):
    """Pack the small parameter matrices into the stationary layouts."""
    W = np.asarray(W, np.float32)
    U = np.asarray(U, np.float32)
    b = np.asarray(b, np.float32)
    Wd = np.asarray(Wd, np.float32)
    bd = np.asarray(bd, np.float32)

    # W2 column (within a seq-pair) = 6*gt + 3*sl + u; sl = seq in pair
    w2 = np.zeros((2 * F, 24), np.float32)
    for gt in range(GATES):
        for sl in range(2):
            for u in range(UNITS):
                w2[sl * F:(sl + 1) * F, 6 * gt + 3 * sl + u] = W[:, 3 * gt + u]

    idm = np.eye(L, dtype=np.float32)
    bdu = np.zeros((L, GATES * L), np.float32)
    for gt in range(GATES):
        ublk = bdu[:, gt * L:(gt + 1) * L]
        for s in range(NS):
            for up in range(UNITS):
                for u in range(UNITS):
                    ublk[3 * s + up, 3 * s + u] = U[up, 3 * gt + u]

    s3 = np.zeros((L, 4 * NB), np.float32)
    for bb in range(NB):
        for c in range(3):
            for u in range(UNITS):
                for d in range(4):
                    s3[9 * bb + 3 * c + u, 4 * bb + d] = Wd[3 * c + u, d]
    bdv = np.tile(bd, NB).reshape(4 * NB, 1).astype(np.float32)

    # phase-1 evacuation bias: stg row r = 32p + 6gt + 3sl + u -> b[3gt+u]
    bev = np.zeros((128, 1), np.float32)
    for p in range(4):
        for gt in range(GATES):
            for sl in range(2):
                for u in range(UNITS):
                    bev[32 * p + 6 * gt + 3 * sl + u, 0] = b[3 * gt + u]

    return {"w2": w2.astype(BF16), "idm": idm.astype(BF16),
            "bdu": bdu.astype(BF16), "s3": s3.astype(BF16),
            "bdv": bdv, "bev": bev}


def _host_xt(inputs, T):
    """[B, T, 192] -> per-core [NS, F, T] bf16 with s = 3*b_local + c."""
    B = inputs.shape[0]
    x = np.asarray(inputs, np.float32).reshape(B, T, 3, F)
    x = np.ascontiguousarray(np.transpose(x, (0, 2, 3, 1)))  # [B, c, F, T]
    x = x.astype(BF16)
    per_core = []
    for k in range(N_CORES):
        per_core.append(x[k * NB:(k + 1) * NB].reshape(NS, F, T))
    return per_core


def kernel(inputs, W, U, b, Wd, bd):
    from concourse.bass_utils import run_bass_kernel_spmd

    B, T, F3 = inputs.shape
    assert (B, T, F3) == (B_FULL, T_FULL, 192)

    key = (T, K_ITERS)
    if key not in _cache:
        _cache[key] = _build_module(T, K_ITERS, debug=False)
    nc = _cache[key]

    consts = _host_consts(W, U, b, Wd, bd, T)
    xts = _host_xt(inputs, T)
    in_maps = [dict(consts, xt=xts[k]) for k in range(N_CORES)]

    global _last_exec_ns
    res = run_bass_kernel_spmd(nc, in_maps, list(range(N_CORES)), trace=TRACE)
    if res.exec_time_ns is not None:
        _last_exec_ns = res.exec_time_ns
    ys = [res.results[k]["y"] for k in range(N_CORES)]  # [32, T] each

    out = np.empty((B, T, 4), np.float32)
    for k in range(N_CORES):
        blk = ys[k].reshape(NB, 4, T)          # [b, d, t]
        out[k * NB:(k + 1) * NB] = np.transpose(blk, (0, 2, 1))
    return out
